# revision 1
# baseline (speedup 1.0000x reference)
"""Multi-head attention (B=4, N=2048, E=1024, H=16, D=64) on 8 TRN2 NeuronCores.

Sharding: core c = (batch b = c//2, head-half hh = c%2). Each core computes,
for its batch, 8 heads worth of Q/K/V projections (a 512-column slice of
Wq/Wk/Wv), full-sequence attention for those heads, and the partial output
projection through the matching 512-row slice of Wo. The host sums the two
partial outputs per batch and adds the closed-form bias correction
(bv/512) @ Wo + bo (each softmax row sums to exactly 1/512 after the
reference's divide-by-E/2).

Host-side prep: x arrives already transposed ([E, N]) and cast to f16, the
weight slices arrive f16, and the Q/K biases arrive as [128, OCH] f32
columns -- so the kernel has no transposes, no input casts, and applies the
bias on the DVE during the PSUM->SBUF copy (tensor_scalar_add) instead of
rank-1 PE matmuls. x^T streams in over two DMA queues (sync/gpsimd) in
token quarters, with the V projection and K chunk 0 interleaved per quarter
so the PE starts within a few us and stays dense.

Layout: Q^T/K^T live [e_out, tok] so the scores are computed transposed
(S^T = K Q^T) with the softmax denominator folded in as a 512-valued
column of V_aug (so the PSUM Z row is already scaled by E/2). exp runs on
ScalarE straight out of PSUM (no max subtraction -- scores are ~N(0,8),
fp32 exp never overflows). Head pairs run concurrently on PE row halves
0-63/64-127 (tile positions via base partitions), sharing one [128, 1024]
S^T PSUM tile so a single exp covers both heads. The exp stream paces the
steady state (~1.1us per key tile); all projection work (K/Q for later
pairs, output projection of finished quarters) is chopped into
single-matmul "filler" closures popped a few per key-tile step, so the PE
never bursts long enough to starve ScalarE.

Per-head normalization: two quick copies free the O/Z PSUM, a rank-1 PE
matmul broadcasts the pre-scaled Z row across 64 partitions,
reciprocal_approx_fast inverts it straight out of PSUM (~5x faster than
DVE reciprocal; Z is a sum of positive exps so the approx edge cases are
unreachable), and the DVE multiply lands the normalized O^T. Drains are
deferred into the next pair's loop. The final quarter's output projection is pre-accumulated over
head chunks 0-2 into SBUF while the last pair runs, so the tail only does
the last chunk's matmul + add + DMA.

Reference quirk handled here: scores are NOT scaled by 1/sqrt(d); the
softmax output is divided by E/2 = 512.
"""

import collections
import sys

if "/opt/trn_rl_repo" not in sys.path:
    sys.path.insert(0, "/opt/trn_rl_repo")

import numpy as np

B, N, E, H = 4, 2048, 1024, 16
D = E // H          # 64
P = 128             # partitions
EH = E // 2         # 512: per-core e_out slice
HL = 8              # heads per core
ECH = E // P        # 8 e_in chunks
OCH = EH // P       # 4 e_out chunks
KC = N // P         # 16 key/token tiles
QH = 4              # q quarters per head pass
QHW = N // QH       # 512
MV = 512            # moving free dim (PSUM bank limit: 512 fp32)
FILL = 3            # filler closures popped per key-tile step
DRAIN_GPSIMD = False  # GpSimd partition_broadcast crashes the exec unit; use PE

_CACHE = {}


def _build():
    import concourse.bass as bass  # noqa: F401  (side-effect imports)
    import concourse.tile as tile
    from concourse import bacc, mybir

    f32 = mybir.dt.float32
    f16 = mybir.dt.float16
    bf16 = mybir.dt.bfloat16
    Exp = mybir.ActivationFunctionType.Exp
    mult = mybir.AluOpType.mult
    add = mybir.AluOpType.add

    nc = bacc.Bacc("TRN2", target_bir_lowering=False, debug=False)

    xt_d = nc.dram_tensor("xt", [E, N], f16, kind="ExternalInput").ap()
    wq_d = nc.dram_tensor("wq", [E, EH], f16, kind="ExternalInput").ap()
    wk_d = nc.dram_tensor("wk", [E, EH], f16, kind="ExternalInput").ap()
    wv_d = nc.dram_tensor("wv", [E, EH], f16, kind="ExternalInput").ap()
    wo_d = nc.dram_tensor("wo", [EH, E], f16, kind="ExternalInput").ap()
    bq_d = nc.dram_tensor("bqc", [P, OCH], f32, kind="ExternalInput").ap()
    bk_d = nc.dram_tensor("bkc", [P, OCH], f32, kind="ExternalInput").ap()
    out_d = nc.dram_tensor("out", [N, E], f32, kind="ExternalOutput").ap()

    with tile.TileContext(nc) as tc:
        with (
            tc.tile_pool(name="persist", bufs=1) as persist,
            tc.tile_pool(name="pt_sb", bufs=8) as pt_sb,
            tc.tile_pool(name="small", bufs=3) as small,
            tc.tile_pool(name="ostage", bufs=4) as ostage,
        ):
            # ---- persistent SBUF tensors (DMA'd directly, all 16-bit) ----
            xT = persist.tile([P, ECH, N], f16, tag="xT")       # x^T
            qT = persist.tile([P, OCH, N], f16, tag="qT")       # (x Wq + bq)^T
            kT = persist.tile([P, OCH, N], f16, tag="kT")
            vaug = persist.tile([P, KC, HL, D + 1], bf16, tag="vaug")
            oT = persist.tile([P, OCH, N], f16, tag="oT")       # normalized O^T
            wq_s = persist.tile([P, ECH, EH], f16, tag="wq_s")
            wk_s = persist.tile([P, ECH, EH], f16, tag="wk_s")
            wv_s = persist.tile([P, ECH, EH], f16, tag="wv_s")
            wo_s = persist.tile([P, OCH, E], f16, tag="wo_s")
            bq_s = persist.tile([P, OCH], f32, tag="bq_s")
            bk_s = persist.tile([P, OCH], f32, tag="bk_s")
            ones64 = persist.tile([1, D], bf16, tag="ones64")

            nc.gpsimd.memset(ones64, 1.0)
            # E/2-valued column of V_aug: the AV matmul's extra output row
            # is then (E/2)*sum(exp) = the softmax denominator pre-scaled
            nc.gpsimd.memset(vaug[:, :, :, D : D + 1], float(E) / 2.0)

            # ---- input DMAs. x^T streams in token quarters over two
            # queues (even chunks on sync, odd on gpsimd); weights on the
            # scalar queue so V-proj/K(0) can start as soon as quarter 0
            # lands ----
            def dma_x(eng, q, parity):
                qsl = slice(q * QHW, (q + 1) * QHW)
                for c in range(parity, ECH, 2):
                    eng.dma_start(
                        out=xT[:, c, qsl], in_=xt_d[c * P : (c + 1) * P, qsl]
                    )

            def dma_w(w_sb, w_dram, nch=ECH):
                for c in range(nch):
                    nc.scalar.dma_start(
                        out=w_sb[:, c, :], in_=w_dram[c * P : (c + 1) * P, :]
                    )

            for q in range(QH):
                dma_x(nc.sync, q, 0)
            dma_x(nc.gpsimd, 0, 1)
            nc.gpsimd.dma_start(out=bk_s, in_=bk_d)
            nc.gpsimd.dma_start(out=bq_s, in_=bq_d)
            dma_x(nc.gpsimd, 1, 1)
            dma_w(wv_s, wv_d)
            dma_w(wk_s, wk_d)
            dma_w(wq_s, wq_d)
            dma_x(nc.scalar, 2, 1)
            dma_x(nc.scalar, 3, 1)
            dma_w(wo_s, wo_d, OCH)

            with (
                tc.tile_pool(name="psS", bufs=2, space="PSUM") as psS,
                tc.tile_pool(name="psO", bufs=2, space="PSUM") as psO,
                tc.tile_pool(name="psF", bufs=2, space="PSUM") as psF,
            ):
                # ---- filler machinery: projection/outproj work chopped
                # into single-PE-op closures, popped FILL per key-tile step
                # so the exp stream never starves behind a PE burst ----
                fillq = collections.deque()  # (req_pair_idx, unit_start, fn)

                def enqueue_unit(req, fns):
                    for k, fn in enumerate(fns):
                        fillq.append((req, k == 0, fn))

                def pop_fill(n=None, upto=None, finish_unit=False):
                    if finish_unit:
                        while fillq and not fillq[0][1]:
                            fillq.popleft()[2]()
                        return
                    if upto is not None:
                        while fillq and fillq[0][0] <= upto:
                            fillq.popleft()[2]()
                        return
                    for _ in range(n):
                        if not fillq:
                            return
                        fillq.popleft()[2]()

                def proj_unit_fns(w_sb, b_sb, dst, co, th):
                    sl = slice(th * MV, (th + 1) * MV)
                    cell = {}

                    def mk(ci):
                        def f():
                            if ci == 0:
                                cell["ps"] = psF.tile(
                                    [P, MV], f32, tag="pf", name="psproj"
                                )
                            nc.tensor.matmul(
                                cell["ps"],
                                lhsT=w_sb[:, ci, co * P : (co + 1) * P],
                                rhs=xT[:, ci, sl],
                                start=(ci == 0),
                                stop=(ci == ECH - 1),
                            )

                        return f

                    fns = [mk(ci) for ci in range(ECH)]

                    def cp():
                        nc.vector.tensor_scalar_add(
                            out=dst[:, co, sl],
                            in0=cell["ps"],
                            scalar1=b_sb[:, co : co + 1],
                        )

                    fns.append(cp)
                    return fns

                def outproj_unit_fns(t, eo, cmax=OCH, partials=None):
                    """Output projection for token tile t, output half eo.
                    With cmax < OCH, accumulates chunks [0, cmax) and stages
                    the partial into SBUF (recorded in `partials`) instead
                    of storing -- the tail then only needs chunk cmax.. ."""
                    DW = 512
                    esl = slice(eo * DW, (eo + 1) * DW)
                    tsl = slice(t * P, (t + 1) * P)
                    cell = {}

                    def mk(c):
                        def f():
                            if c == 0:
                                cell["ps"] = psF.tile(
                                    [P, DW], f32, tag="pf", name="psout"
                                )
                            nc.tensor.matmul(
                                cell["ps"],
                                lhsT=oT[:, c, tsl],
                                rhs=wo_s[:, c, esl],
                                start=(c == 0),
                                stop=(c == cmax - 1),
                            )

                        return f

                    fns = [mk(c) for c in range(cmax)]

                    if cmax < OCH:

                        def cp():
                            op = ostage.tile(
                                [P, DW], f32, tag="opart", bufs=8, name="opart"
                            )
                            nc.vector.tensor_copy(out=op, in_=cell["ps"])
                            partials[(t, eo)] = op

                    else:

                        def cp():
                            os_ = ostage.tile([P, DW], f32, tag="os")
                            nc.vector.tensor_copy(out=os_, in_=cell["ps"])
                            nc.sync.dma_start(out=out_d[tsl, esl], in_=os_)

                    fns.append(cp)
                    return fns

                q3partials = {}

                def enqueue_outproj(qq):
                    for t in range(qq * (KC // QH), (qq + 1) * (KC // QH)):
                        for eo in range(2):
                            enqueue_unit(10**6, outproj_unit_fns(t, eo))

                def enqueue_outproj_partial(qq):
                    for t in range(qq * (KC // QH), (qq + 1) * (KC // QH)):
                        for eo in range(2):
                            enqueue_unit(
                                10**6,
                                outproj_unit_fns(
                                    t, eo, cmax=OCH - 1, partials=q3partials
                                ),
                            )

                # ---- prefix: V projection and K(0) interleaved per token
                # quarter as its x^T lands, then Q(quarter 0, chunk 0) ----
                def vproj(t):
                    pv = psF.tile([P, EH], f32, tag="pf", name="pv")
                    for ci in range(ECH):
                        nc.tensor.matmul(
                            pv,
                            lhsT=xT[:, ci, t * P : (t + 1) * P],
                            rhs=wv_s[:, ci, :],
                            start=(ci == 0),
                            stop=(ci == ECH - 1),
                        )
                    nc.vector.tensor_copy(
                        out=vaug[:, t, :, 0:D],
                        in_=pv.rearrange("p (h d) -> p h d", h=HL),
                    )

                def vproj_unit_fns(t):
                    cell = {}

                    def mk(ci):
                        def f():
                            if ci == 0:
                                cell["ps"] = psF.tile(
                                    [P, EH], f32, tag="pf", name="pv"
                                )
                            nc.tensor.matmul(
                                cell["ps"],
                                lhsT=xT[:, ci, t * P : (t + 1) * P],
                                rhs=wv_s[:, ci, :],
                                start=(ci == 0),
                                stop=(ci == ECH - 1),
                            )

                        return f

                    fns = [mk(ci) for ci in range(ECH)]

                    def cp():
                        nc.vector.tensor_copy(
                            out=vaug[:, t, :, 0:D],
                            in_=cell["ps"].rearrange("p (h d) -> p h d", h=HL),
                        )

                    fns.append(cp)
                    return fns

                for q in range(QH):
                    for t in range(4 * q, min(4 * q + 4, 13)):
                        vproj(t)
                    if q < 3:
                        for fn in proj_unit_fns(wk_s, bk_s, kT, 0, q):
                            fn()
                    if q == 1:
                        for fn in proj_unit_fns(wq_s, bq_s, qT, 0, 0):
                            fn()
                # the rest of pair (0,0)'s needs drain as the first fillers:
                # K(0) tokens 1536-2048 (used from step 10), then V13-15
                # (used at steps 13-15) -- FILL=3/step clears them in time
                enqueue_unit(0, proj_unit_fns(wk_s, bk_s, kT, 0, 3))
                for t in range(13, KC):
                    enqueue_unit(0, vproj_unit_fns(t))

                # remaining projections become fillers, FIFO in deadline
                # order: K(j)/Q(0,j) before pair (0,j), Q(qq,j) before
                # pair 4*qq+j
                for j in range(1, HL // 2):
                    for th in range(N // MV):
                        enqueue_unit(j, proj_unit_fns(wk_s, bk_s, kT, j, th))
                    enqueue_unit(j, proj_unit_fns(wq_s, bq_s, qT, j, 0))
                for qq in range(1, QH):
                    for j in range(HL // 2):
                        enqueue_unit(
                            4 * qq + j, proj_unit_fns(wq_s, bq_s, qT, j, qq)
                        )

                def s_pair_for(j, qq, kc):
                    qsl = slice(qq * QHW, (qq + 1) * QHW)
                    ss = psS.tile([P, 2 * QHW], f32, tag="ss")
                    ksl = slice(kc * P, (kc + 1) * P)
                    nc.tensor.matmul(
                        ss[:, 0:QHW],
                        lhsT=kT[0:D, j, ksl],
                        rhs=qT[0:D, j, qsl],
                        start=True,
                        stop=True,
                    )
                    nc.tensor.matmul(
                        ss[:, QHW : 2 * QHW],
                        lhsT=kT[D : 2 * D, j, ksl],
                        rhs=qT[D : 2 * D, j, qsl],
                        start=True,
                        stop=True,
                    )
                    return ss

                def drain_head(h, ocp, zrow, pqq):
                    """Normalize one head's accumulated O^T into oT. The
                    pre-scaled Z row is broadcast across 64 partitions
                    (GpSimd partition_broadcast from a partition-0 source,
                    or a rank-1 PE matmul), inverted with the fast-approx
                    reciprocal, and the DVE multiply writes oT. Emitted
                    deep inside the NEXT pair's loop."""
                    bp = (h % 2) * D
                    qsl = slice(pqq * QHW, (pqq + 1) * QHW)
                    zinv = small.tile([D, QHW], f32, tag="zinv")
                    if DRAIN_GPSIMD:
                        zb = small.tile([D, QHW], f32, tag="zb")
                        nc.gpsimd.partition_broadcast(zb, zrow, channels=D)
                        nc.vector.reciprocal_approx_fast(out=zinv, in_=zb)
                    else:
                        pob = psF.tile([P, QHW], f32, tag="pf", name="pob")
                        nc.tensor.matmul(
                            pob[0:D, :],
                            lhsT=ones64,
                            rhs=zrow,
                            start=True,
                            stop=True,
                        )
                        nc.vector.reciprocal_approx_fast(
                            out=zinv, in_=pob[0:D, :]
                        )
                    nc.vector.tensor_tensor(
                        out=oT[bp : bp + D, h // 2, qsl],
                        in0=ocp,
                        in1=zinv,
                        op=mult,
                    )

                def attn_pair(
                    j, qq, pending, preS, nxt, nxt_idx, after_drain, steps_left
                ):
                    def fill_rate(kc):
                        return FILL
                    """S^T/exp/O for heads (2j, 2j+1) on quarter qq. S-pairs
                    run two steps ahead of the O-pairs (and preload into the
                    NEXT pair at kc 14/15) so ScalarE's exp stream never
                    waits on the PE's static order; the previous pair's
                    normalization drains mid-loop; filler closures soak up
                    the per-step PE slack."""
                    po_e = psO.tile([P, QHW], f32, tag="po")
                    po_o = psO.tile([P, QHW], f32, tag="po")
                    sss = (
                        preS
                        if preS is not None
                        else [s_pair_for(j, qq, 0), s_pair_for(j, qq, 1)]
                    )
                    nxtS = []
                    pts = []

                    def av_pair(kc):
                        nc.tensor.matmul(
                            po_e[0 : D + 1, :],
                            lhsT=vaug[:, kc, 2 * j, :],
                            rhs=pts[kc][:, 0:QHW],
                            start=(kc == 0),
                            stop=(kc == KC - 1),
                        )
                        nc.tensor.matmul(
                            po_o[0 : D + 1, :],
                            lhsT=vaug[:, kc, 2 * j + 1, :],
                            rhs=pts[kc][:, QHW : 2 * QHW],
                            start=(kc == 0),
                            stop=(kc == KC - 1),
                        )

                    for kc in range(KC):
                        pT = pt_sb.tile([P, 2 * QHW], bf16, tag="pT")
                        nc.scalar.activation(pT, sss[kc], Exp)
                        pts.append(pT)
                        if kc + 2 < KC:
                            sss.append(s_pair_for(j, qq, kc + 2))
                        # AV lags exp by one step: its pT finished a full
                        # step ago (no ScalarE sem wait), and the first AV
                        # of a pair lands after the previous pair's PSUM-
                        # freeing copies have drained (no psO WAR stall)
                        if kc >= 1:
                            av_pair(kc - 1)
                        if kc == 5 and pending:
                            if not DRAIN_GPSIMD:
                                # psF slot discipline: finish any half-
                                # emitted filler unit before pob allocs
                                pop_fill(finish_unit=True)
                            for args in pending:
                                drain_head(*args)
                            pending.clear()
                            if after_drain is not None:
                                after_drain()
                        if nxt is not None and kc >= KC - 2:
                            if kc == KC - 2:
                                # anything the next pair depends on must be
                                # emitted before its S tiles start
                                pop_fill(upto=nxt_idx)
                            nq, njj = nxt
                            nxtS.append(s_pair_for(njj, nq, kc - (KC - 2)))
                        pop_fill(fill_rate(kc))
                    av_pair(KC - 1)
                    out = []
                    for h, po in ((2 * j, po_e), (2 * j + 1, po_o)):
                        # two quick copies free the PSUM accumulator;
                        # Z first so the drain can start early
                        zrow = small.tile(
                            [1, QHW], f32 if DRAIN_GPSIMD else bf16, tag="zrow"
                        )
                        nc.vector.tensor_copy(out=zrow, in_=po[D : D + 1, :])
                        ocp = small.tile([D, QHW], bf16, tag="ocp")
                        nc.vector.tensor_copy(out=ocp, in_=po[0:D, :])
                        out.append((h, ocp, zrow, qq))
                    return out, nxtS

                pairs = [(qq, j) for qq in range(QH) for j in range(HL // 2)]
                pending, preS = [], None
                for idx, (qq, j) in enumerate(pairs):
                    nxt = pairs[idx + 1] if idx + 1 < len(pairs) else None
                    after_drain = None
                    if j == 0 and qq >= 1:
                        after_drain = (lambda q=qq - 1: enqueue_outproj(q))
                    elif (qq, j) == (QH - 1, HL // 2 - 1):
                        # last pair: pre-accumulate quarter 3's outproj over
                        # head chunks 0-2 while this pair runs
                        after_drain = (lambda: enqueue_outproj_partial(QH - 1))
                    pending, preS = attn_pair(
                        j, qq, pending, preS, nxt, idx + 1, after_drain,
                        (len(pairs) - idx) * KC,
                    )
                pop_fill(upto=10**6)
                for args in pending:
                    drain_head(*args)
                pending.clear()
                # tail: only the last head chunk's matmul + staged-partial
                # add + store for quarter 3
                DW = 512
                for t in range((QH - 1) * (KC // QH), QH * (KC // QH)):
                    for eo in range(2):
                        esl = slice(eo * DW, (eo + 1) * DW)
                        tsl = slice(t * P, (t + 1) * P)
                        pod = psF.tile([P, DW], f32, tag="pf", name="podl")
                        nc.tensor.matmul(
                            pod,
                            lhsT=oT[:, OCH - 1, tsl],
                            rhs=wo_s[:, OCH - 1, esl],
                            start=True,
                            stop=True,
                        )
                        os_ = ostage.tile([P, DW], f32, tag="os")
                        nc.vector.tensor_tensor(
                            out=os_, in0=pod, in1=q3partials[(t, eo)], op=add
                        )
                        nc.sync.dma_start(out=out_d[tsl, esl], in_=os_)
    nc.compile()
    return nc


def _get_nc():
    if "nc" not in _CACHE:
        _CACHE["nc"] = _build()
    return _CACHE["nc"]


def _make_in_maps(x, Wq, bq, Wk, bk, Wv, bv, Wo, bo):
    x = np.asarray(x, dtype=np.float32)
    Wq = np.asarray(Wq, dtype=np.float16)
    Wk = np.asarray(Wk, dtype=np.float16)
    Wv = np.asarray(Wv, dtype=np.float16)
    Wo = np.asarray(Wo, dtype=np.float16)
    bq = np.asarray(bq, dtype=np.float32)
    bk = np.asarray(bk, dtype=np.float32)

    xts = [np.ascontiguousarray(x[b].T.astype(np.float16)) for b in range(B)]
    in_maps = []
    for c in range(8):
        b, hh = divmod(c, 2)
        sl = slice(hh * EH, (hh + 1) * EH)
        in_maps.append(
            {
                "xt": xts[b],
                "wq": np.ascontiguousarray(Wq[:, sl]),
                "wk": np.ascontiguousarray(Wk[:, sl]),
                "wv": np.ascontiguousarray(Wv[:, sl]),
                "wo": np.ascontiguousarray(Wo[sl, :]),
                "bqc": np.ascontiguousarray(bq[sl].reshape(OCH, P).T),
                "bkc": np.ascontiguousarray(bk[sl].reshape(OCH, P).T),
            }
        )
    return in_maps


def kernel(x, Wq, bq, Wk, bk, Wv, bv, Wo, bo):
    from concourse.bass_utils import run_bass_kernel_spmd

    Wo32 = np.asarray(Wo, dtype=np.float32)
    bv32 = np.asarray(bv, dtype=np.float32)
    bo32 = np.asarray(bo, dtype=np.float32)

    nc = _get_nc()
    in_maps = _make_in_maps(x, Wq, bq, Wk, bk, Wv, bv, Wo, bo)
    res = run_bass_kernel_spmd(nc, in_maps, list(range(8))).results

    # Exact bias correction: softmax rows sum to 1, so A rows sum to 1/512
    # and the V-bias term is the constant row (bv/512) @ Wo; bo likewise.
    corr = (
        bv32.astype(np.float64) @ Wo32.astype(np.float64) / (E / 2.0)
        + bo32.astype(np.float64)
    ).astype(np.float32)

    out = np.empty((B, N, E), dtype=np.float32)
    for b in range(B):
        out[b] = res[2 * b]["out"] + res[2 * b + 1]["out"] + corr[None, :]
    return out

